# revision 1
# baseline (speedup 1.0000x reference)
"""Trainium2 Bass kernel: Conv2d [8,8,1024,1024] x [8,8,3,3] (+bias), with
the reference's roll-by-1 on H, VALID padding -> [8,8,1022,1022].

Strategy: data-parallel over the batch dim (1 image per NeuronCore, 8 cores).
Per core the conv runs as blocked matmuls on the tensor engine:
  - SBUF input tile [128, W]: partition p = q*8 + cin holds (rolled) input row
    s = 14*b + q of channel cin (16 rows x 8 cin = 128 partitions).
  - lhsT [128, 112]: zero-padded weights; column m = dx*8 + co holds
    filt[co, cin, q-dx, j] at partition (q, cin) when 0 <= q-dx <= 2.
    M packs 14 output rows x 8 couts (dx-major: the output DMA's outer
    HBM dim is then dx=14, fanning across 14 SDMA engines).
  - The 3 W-taps (j) are 3 accumulating matmuls whose rhs is the same tile
    shifted by j in the free dim. dtype float32r (fast fp32 PE path).
  - PSUM [112, 511] is evicted by ScalarE activation(Identity, bias) into
    SBUF, then DMA'd out. The H-roll is folded into the input DMA rows.
"""

import os
import sys

for _p in ("/opt/trn_rl_repo",):
    if _p not in sys.path and os.path.isdir(_p):
        sys.path.insert(0, _p)

import numpy as np

import concourse.bacc as bacc
import concourse.bass as bass
import concourse.mybir as mybir
from concourse.bass_utils import run_bass_kernel_spmd
from concourse.tile import TileContext

F32 = mybir.dt.float32
F32R = mybir.dt.float32r

N_CORES = 8
CIN = 8
COUT = 8
KH = 3
KW = 3


def _pad32(n):
    return (n + 31) // 32 * 32


def _const_layout(D, d_last):
    """Column offsets in the packed consts array."""
    M = COUT * D
    off_bias = KW * M
    cw = off_bias + 1
    off_wl = off_bl = None
    if d_last:
        Ml = COUT * d_last
        off_wl = cw
        off_bl = off_wl + KW * Ml
        cw = off_bl + 1
    return dict(M=M, off_bias=off_bias, off_wl=off_wl, off_bl=off_bl,
                cw=cw, cw_pad=_pad32(cw))


def build_nc(
    H: int = 1024,
    W: int = 1024,
    D: int = 14,
    in_bufs: int = 16,
    out_bufs: int = 8,
    psum_bufs: int = 8,
):
    """Build the per-core Bass program. Returns (nc, meta)."""
    HOUT = H - (KH - 1)
    WOUT = W - (KW - 1)
    R = D + 2  # input rows resident per block
    assert R * CIN <= 128
    n_full = HOUT // D
    d_last = HOUT - n_full * D  # partial last block (0 for 1022/14)
    lay = _const_layout(D, d_last)
    M = lay["M"]
    # W chunks of <= 512, even (fp32r needs even N and wants N >= 256;
    # PSUM bank holds 512 fp32)
    assert WOUT % 2 == 0
    chunks = []
    w0 = 0
    while w0 < WOUT:
        n = min(512, WOUT - w0)
        if n % 2:
            n -= 1
        chunks.append((w0, n))
        w0 += n

    nc = bacc.Bacc("TRN2", target_bir_lowering=False, debug=False,
                   num_devices=N_CORES)
    inp_d = nc.dram_tensor("inp", [CIN, H, W], F32R, kind="ExternalInput")
    consts_d = nc.dram_tensor("consts", [128, lay["cw_pad"]], F32R,
                              kind="ExternalInput")
    out_d = nc.dram_tensor("out", [COUT, HOUT, WOUT], F32, kind="ExternalOutput")

    ident = mybir.ActivationFunctionType.Identity

    with TileContext(nc) as tc:
        with (
            tc.tile_pool(name="win", bufs=1) as wpool,
            tc.tile_pool(name="inp", bufs=in_bufs) as ipool,
            tc.tile_pool(name="outp", bufs=out_bufs) as opool,
            tc.tile_pool(name="ps", bufs=psum_bufs, space="PSUM") as ppool,
        ):
            cw_t = wpool.tile([128, lay["cw_pad"]], F32R, tag="consts")
            nc.sync.dma_start(out=cw_t[:], in_=consts_d[:])
            bias_t = cw_t[0:M, lay["off_bias"]:lay["off_bias"] + 1].bitcast(F32)
            if d_last:
                Ml = COUT * d_last
                bl_t = cw_t[0:Ml, lay["off_bl"]:lay["off_bl"] + 1].bitcast(F32)

            n_blocks = n_full + (1 if d_last else 0)
            for b in range(n_blocks):
                last = d_last and b == n_full
                Db = d_last if last else D
                Rb = Db + 2
                Mb = COUT * Db
                Kb = Rb * CIN

                t_full = ipool.tile([128, _pad32(W)], F32R, tag="inp")
                t = t_full[:, 0:W]
                # rolled input: row s of the rolled image = inp row (s-1)%H;
                # block b needs rolled rows [D*b, D*b+Rb) on partitions
                # p = q*CIN + c  (q = row-in-block, c = cin)
                r0 = D * b - 1
                if b == 0:
                    nc.sync.dma_start(
                        out=t[CIN:Rb * CIN, :],
                        in_=inp_d[:, 0:Rb - 1, :].rearrange("c q w -> q c w"))
                    nc.sync.dma_start(
                        out=t[0:CIN, :],
                        in_=inp_d[:, H - 1:H, :].rearrange("c q w -> q c w"))
                else:
                    nc.sync.dma_start(
                        out=t[0:Rb * CIN, :],
                        in_=inp_d[:, r0:r0 + Rb, :].rearrange("c q w -> q c w"))

                ot_full = opool.tile([M, _pad32(WOUT)], F32, tag="outp")
                ot = ot_full[:, 0:WOUT]
                for (c0, n) in chunks:
                    ps = ppool.tile([Mb, n], F32, tag="ps")
                    for j in range(KW):
                        if last:
                            lhsT = cw_t[0:Kb,
                                        lay["off_wl"] + j * Mb:
                                        lay["off_wl"] + (j + 1) * Mb]
                        else:
                            lhsT = cw_t[:, j * M:(j + 1) * M]
                        nc.tensor.matmul(
                            ps[:],
                            lhsT=lhsT,
                            rhs=t[0:Kb, c0 + j:c0 + j + n],
                            start=(j == 0),
                            stop=(j == KW - 1),
                        )
                    nc.vector.tensor_scalar_add(
                        ot[0:Mb, c0:c0 + n], ps[:],
                        (bl_t if last else bias_t))
                nc.scalar.dma_start(
                    out=out_d[:, D * b:D * b + Db, :].rearrange(
                        "co x w -> x co w"),
                    in_=ot[0:Mb, :])

    nc.compile()
    meta = dict(H=H, W=W, D=D, HOUT=HOUT, WOUT=WOUT, d_last=d_last, lay=lay)
    return nc, meta


def _fill_wmat(wmat, filt, D, col0):
    """wmat[q*CIN+c, col0 + j*COUT*D + co*D + dx] = filt[co, c, q-dx, j]."""
    Md = COUT * D
    for j in range(KW):
        for q in range(D + 2):
            for dx in range(D):
                i = q - dx
                if 0 <= i < KH:
                    for c in range(CIN):
                        wmat[q * CIN + c,
                             col0 + j * Md + dx * COUT + np.arange(COUT)] = \
                            filt[:, c, i, j]


def make_consts(filt: np.ndarray, bias: np.ndarray, D: int, d_last: int):
    """Host-side prep of filter+bias into the packed SBUF consts layout."""
    lay = _const_layout(D, d_last)
    consts = np.zeros((128, lay["cw_pad"]), np.float32)
    _fill_wmat(consts, filt, D, 0)
    consts[0:COUT * D, lay["off_bias"]] = np.tile(bias, D)
    if d_last:
        _fill_wmat(consts, filt, d_last, lay["off_wl"])
        consts[0:COUT * d_last, lay["off_bl"]] = np.tile(bias, d_last)
    return consts


_CACHE = {}


def _get_nc():
    if "nc" not in _CACHE:
        _CACHE["nc"] = build_nc()
    return _CACHE["nc"]


def kernel(inp: np.ndarray, filt: np.ndarray, bias: np.ndarray) -> np.ndarray:
    inp = np.asarray(inp, np.float32)
    filt = np.asarray(filt, np.float32)
    bias = np.asarray(bias, np.float32)
    nc, meta = _get_nc()
    consts = make_consts(filt, bias, meta["D"], meta["d_last"])
    in_maps = [
        {"inp": np.ascontiguousarray(inp[n]), "consts": consts}
        for n in range(N_CORES)
    ]
    res = run_bass_kernel_spmd(nc, in_maps, list(range(N_CORES)))
    out = np.stack([res.results[c]["out"] for c in range(N_CORES)], axis=0)
    return out



# revision 3
# speedup vs baseline: 1.5745x; 1.5745x over previous
"""Trainium2 Bass kernel: Conv2d [8,8,1024,1024] x [8,8,3,3] (+bias), with
the reference's roll-by-1 on H, VALID padding -> [8,8,1022,1022].

Strategy: data-parallel over the batch dim (1 image per NeuronCore, 8 cores).
The kernel is HBM-bandwidth bound, so everything is shaped to (a) halve the
bytes and (b) double the per-partition DMA descriptor size:

  - All HBM tensors are bf16 (tolerance is 2e-2; bf16 end-to-end is ~4e-3).
    The host pre-casts, prepends the rolled row (input becomes [8,1025,1024]
    so no wraparound DMA), and upcasts the output back to f32.
  - Input SBUF tile [128, 2048]: partition p = c*16+g holds TWO consecutive
    rows (2g, 2g+1 of the block) of channel c -> one 4KB descriptor per
    partition instead of two 2KB ones.
  - A block covers D=30 output rows (R=32 input rows).  Outputs are computed
    in two parity groups G in {0,1}: group G covers rows x0+2t+G, packed
    m = co*15+t (M=120).  Per (G, W-chunk) the conv is 6 accumulating
    matmuls: u in {0,1} selects the row-half of the input tile (rhs free
    offset u*1024), j in {0,1,2} the W-tap; lhsT[(c,g),(co,t)] holds
    filt[co,c,i,j] with i = 2(g-t)+(u-G) when 0<=i<3.
  - After the parity groups are evicted (ScalarE/VectorE adds bias, casts to
    bf16) into ot[m, G*1022 + y], partition m holds output rows x0+2t and
    x0+2t+1 -> contiguous 4088B HBM write descriptors.
  - 34 full blocks + one D=2 tail block (1022 = 34*30 + 2).
"""

import os
import sys

for _p in ("/opt/trn_rl_repo",):
    if _p not in sys.path and os.path.isdir(_p):
        sys.path.insert(0, _p)

import ml_dtypes
import numpy as np

import concourse.bacc as bacc
import concourse.bass as bass
import concourse.mybir as mybir
from concourse.bass_utils import run_bass_kernel_spmd
from concourse.tile import TileContext

F32 = mybir.dt.float32
BF16 = mybir.dt.bfloat16
NP_BF16 = ml_dtypes.bfloat16

N_CORES = 8
CIN = 8
COUT = 8
KH = 3
KW = 3
H = 1024
W = 1024
HOUT = H - 2
WOUT = W - 2
D = 30               # output rows per full block
T = D // 2           # row-pairs per full block
N_FULL = HOUT // D   # 34
T_LAST = (HOUT - N_FULL * D) // 2  # 1
CHUNKS = [(0, 512), (512, 510)]

NW_FULL = 12 * COUT * T      # 12 weight mats [128, 120]
NW_LAST = 12 * COUT * T_LAST
NW = NW_FULL + NW_LAST


def _woff(G, u, j, Tb):
    base = 0 if Tb == T else NW_FULL
    return base + ((G * 2 + u) * KW + j) * COUT * Tb


def build_nc(in_bufs: int = 12, out_bufs: int = 8, psum_bufs: int = 8):
    nc = bacc.Bacc("TRN2", target_bir_lowering=False, debug=False,
                   num_devices=N_CORES)
    inp_d = nc.dram_tensor("inp", [CIN, H + 1, W], BF16, kind="ExternalInput")
    w_d = nc.dram_tensor("wconst", [128, NW], BF16, kind="ExternalInput")
    b_d = nc.dram_tensor("bconst", [128, 2], F32, kind="ExternalInput")
    out_d = nc.dram_tensor("out", [COUT, HOUT, WOUT], BF16,
                           kind="ExternalOutput")

    with TileContext(nc) as tc:
        with (
            tc.tile_pool(name="win", bufs=1) as wpool,
            tc.tile_pool(name="inp", bufs=in_bufs) as ipool,
            tc.tile_pool(name="outp", bufs=out_bufs) as opool,
            tc.tile_pool(name="ps", bufs=psum_bufs, space="PSUM") as ppool,
        ):
            wt = wpool.tile([128, NW], BF16, tag="wt")
            bt = wpool.tile([128, 2], F32, tag="bt")
            nc.sync.dma_start(out=wt[:], in_=w_d[:])
            nc.sync.dma_start(out=bt[:], in_=b_d[:])

            for b in range(N_FULL + 1):
                Tb = T if b < N_FULL else T_LAST
                x0 = D * b
                Kb = CIN * (Tb + 1)
                Mb = COUT * Tb

                t = ipool.tile([128, 2 * W], BF16, tag="t")
                nc.sync.dma_start(
                    out=t[0:Kb, :],
                    in_=inp_d[:, x0:x0 + 2 * Tb + 2, :].rearrange(
                        "c (g p) w -> c g (p w)", p=2))

                ot = opool.tile([COUT * T, 2 * W], BF16, tag="ot")
                for G in range(2):
                    bias_ap = (bt[0:Mb, 0:1] if Tb == T
                               else bt[0:Mb, 1:2])
                    for (c0, n) in CHUNKS:
                        ps = ppool.tile([Mb, n], F32, tag="ps")
                        k = 0
                        for u in range(2):
                            for j in range(KW):
                                lhsT = wt[0:Kb,
                                          _woff(G, u, j, Tb):
                                          _woff(G, u, j, Tb) + Mb]
                                nc.tensor.matmul(
                                    ps[:],
                                    lhsT=lhsT,
                                    rhs=t[0:Kb, u * W + c0 + j:
                                          u * W + c0 + j + n],
                                    start=(k == 0),
                                    stop=(k == 2 * KW - 1),
                                )
                                k += 1
                        nc.vector.tensor_scalar_add(
                            ot[0:Mb, G * WOUT + c0:G * WOUT + c0 + n],
                            ps[:], bias_ap)
                nc.scalar.dma_start(
                    out=out_d[:, x0:x0 + 2 * Tb, :].rearrange(
                        "co (t p) w -> co t (p w)", p=2),
                    in_=ot[0:Mb, 0:2 * WOUT])

    nc.compile()
    return nc


def make_consts(filt: np.ndarray, bias: np.ndarray):
    """Pack the 2x 12 banded weight matrices (bf16) + bias columns (f32)."""
    wconst = np.zeros((128, NW), np.float32)
    for Tb in (T, T_LAST):
        for G in range(2):
            for u in range(2):
                for j in range(KW):
                    col0 = _woff(G, u, j, Tb)
                    for g in range(Tb + 1):
                        for t in range(Tb):
                            i = 2 * (g - t) + (u - G)
                            if 0 <= i < KH:
                                for c in range(CIN):
                                    wconst[c * (Tb + 1) + g,
                                           col0 + np.arange(COUT) * Tb + t] = \
                                        filt[:, c, i, j]
    bconst = np.zeros((128, 2), np.float32)
    bconst[0:COUT * T, 0] = np.repeat(bias, T)
    bconst[0:COUT * T_LAST, 1] = np.repeat(bias, T_LAST)
    return wconst.astype(NP_BF16), bconst


_CACHE = {}


def _get_nc():
    if "nc" not in _CACHE:
        _CACHE["nc"] = build_nc()
    return _CACHE["nc"]


def make_in_maps(inp, filt, bias):
    wconst, bconst = make_consts(filt, bias)
    inp_b = inp.astype(NP_BF16)
    in_maps = []
    for n in range(N_CORES):
        core = np.concatenate([inp_b[n][:, -1:, :], inp_b[n]], axis=1)
        in_maps.append({
            "inp": np.ascontiguousarray(core),
            "wconst": wconst,
            "bconst": bconst,
        })
    return in_maps


def kernel(inp: np.ndarray, filt: np.ndarray, bias: np.ndarray) -> np.ndarray:
    inp = np.asarray(inp, np.float32)
    filt = np.asarray(filt, np.float32)
    bias = np.asarray(bias, np.float32)
    nc = _get_nc()
    in_maps = make_in_maps(inp, filt, bias)
    res = run_bass_kernel_spmd(nc, in_maps, list(range(N_CORES)))
    out = np.stack([res.results[c]["out"] for c in range(N_CORES)], axis=0)
    return out.astype(np.float32)


# revision 6
# speedup vs baseline: 4.4023x; 2.7961x over previous
"""Trainium2 Bass kernel: Conv2d [8,8,1024,1024] x [8,8,3,3] (+bias), with
the reference's roll-by-1 on H, VALID padding -> [8,8,1022,1022].

Strategy: data-parallel over the batch dim (1 image per NeuronCore, 8 cores).
The kernel is HBM-bandwidth bound; all layout work is pushed to the host so
the device sees only large dense DMAs:

  - Everything on HBM is bf16 (tolerance is 2e-2; bf16 end-to-end is ~4e-3).
  - The host pre-stages the input into the exact SBUF tile layout
    `staged_in[128, 73*1024]`: partition p = c*16+q holds row 14b+q-1 (the
    roll is folded in) of channel c for block b at columns [1024b, 1024b+1024).
    Blocks are fetched 8-9 at a time -> one dense 16-18KB descriptor per
    partition per dma_start (vs 2-4KB strided descriptors straight from NCHW,
    which run at ~half rate and pay a per-dma_start completion stall).
  - Per block the conv is 3 accumulating matmuls (one per W-tap j) with the
    banded weight lhsT[(c,q),(co,dx)] = filt[co,c,q-dx,j]; K=128 = 16 rows x
    8 cin, M=112 = 14 output rows x 8 cout, N = the W dim in chunks 512+510.
  - PSUM is evicted (+bias, cast to bf16) alternately by VectorE and ScalarE
    into ot[112, g*1022]; one dense dma_start per group writes
    staged_out[112, 73*1022].  The host reassembles [8,1022,1022] f32.
"""

import os
import sys

for _p in ("/opt/trn_rl_repo",):
    if _p not in sys.path and os.path.isdir(_p):
        sys.path.insert(0, _p)

import ml_dtypes
import numpy as np

import concourse.bacc as bacc
import concourse.bass as bass
import concourse.mybir as mybir
from concourse.bass_utils import run_bass_kernel_spmd
from concourse.tile import TileContext

F32 = mybir.dt.float32
BF16 = mybir.dt.bfloat16
NP_BF16 = ml_dtypes.bfloat16

N_CORES = 8
CIN = 8
COUT = 8
KH = 3
KW = 3
H = 1024
W = 1024
HOUT = H - 2
WOUT = W - 2
D = 14                # output rows per block
R = D + 2             # input rows per block
NB = HOUT // D        # 73 blocks, exact
M = COUT * D          # 112
CHUNKS = [(0, 512), (512, 510)]
GROUPS = [(b0, 8) for b0 in range(0, 64, 8)] + [(64, 9)]  # 8*8+9 = 73
GMAX = 9


def build_nc(in_bufs: int = 3, out_bufs: int = 3, psum_bufs: int = 8):
    nc = bacc.Bacc("TRN2", target_bir_lowering=False, debug=False,
                   num_devices=N_CORES)
    in_d = nc.dram_tensor("staged_in", [128, NB * W], BF16,
                          kind="ExternalInput")
    w_d = nc.dram_tensor("wconst", [128, KW * M], BF16, kind="ExternalInput")
    b_d = nc.dram_tensor("bconst", [M, 1], F32, kind="ExternalInput")
    out_d = nc.dram_tensor("staged_out", [M, NB * WOUT], BF16,
                           kind="ExternalOutput")

    with TileContext(nc) as tc:
        with (
            tc.tile_pool(name="win", bufs=1) as wpool,
            tc.tile_pool(name="inp", bufs=in_bufs) as ipool,
            tc.tile_pool(name="outp", bufs=out_bufs) as opool,
            tc.tile_pool(name="ps", bufs=psum_bufs, space="PSUM") as ppool,
        ):
            wt = wpool.tile([128, KW * M], BF16, tag="wt")
            bt = wpool.tile([M, 1], F32, tag="bt")
            nc.sync.dma_start(out=wt[:], in_=w_d[:])
            nc.sync.dma_start(out=bt[:], in_=b_d[:])

            ev = 0
            for (b0, g) in GROUPS:
                t = ipool.tile([128, GMAX * W], BF16, tag="t")
                nc.sync.dma_start(
                    out=t[0:128, 0:g * W],
                    in_=in_d[:, b0 * W:(b0 + g) * W])
                ot = opool.tile([M, GMAX * WOUT], BF16, tag="ot")
                for i in range(g):
                    for (c0, n) in CHUNKS:
                        ps = ppool.tile([M, 512], F32, tag="ps")
                        for j in range(KW):
                            nc.tensor.matmul(
                                ps[0:M, 0:n],
                                lhsT=wt[:, j * M:(j + 1) * M],
                                rhs=t[0:128, i * W + c0 + j:
                                      i * W + c0 + j + n],
                                start=(j == 0),
                                stop=(j == KW - 1),
                            )
                        dst = ot[0:M, i * WOUT + c0:i * WOUT + c0 + n]
                        if ev % 2 == 0:
                            nc.vector.tensor_scalar_add(dst, ps[0:M, 0:n],
                                                        bt[:])
                        else:
                            nc.scalar.add(dst, ps[0:M, 0:n], bt[:])
                        ev += 1
                nc.scalar.dma_start(
                    out=out_d[:, b0 * WOUT:(b0 + g) * WOUT],
                    in_=ot[0:M, 0:g * WOUT])

    nc.compile()
    return nc


def make_consts(filt: np.ndarray, bias: np.ndarray):
    wconst = np.zeros((128, KW * M), np.float32)
    for j in range(KW):
        for q in range(R):
            for dx in range(D):
                i = q - dx
                if 0 <= i < KH:
                    for c in range(CIN):
                        wconst[c * R + q,
                               j * M + np.arange(COUT) * D + dx] = \
                            filt[:, c, i, j]
    bconst = np.repeat(bias, D).astype(np.float32)[:, None]
    return wconst.astype(NP_BF16), bconst


_CACHE = {}


def _get_nc():
    if "nc" not in _CACHE:
        _CACHE["nc"] = build_nc()
    return _CACHE["nc"]


def _stage_input(core_bf16: np.ndarray) -> np.ndarray:
    """[8,1024,1024] bf16 -> staged [128, 73*1024]: partition c*16+q, block b
    holds rolled row 14b+q = orig row (14b+q-1) mod 1024."""
    dev = np.concatenate([core_bf16[:, -1:, :], core_bf16], axis=1)  # 1025 rows
    s = np.lib.stride_tricks.as_strided(
        dev,
        shape=(CIN, R, NB, W),
        strides=(dev.strides[0], dev.strides[1], D * dev.strides[1],
                 dev.strides[2]))
    return np.ascontiguousarray(s.reshape(CIN * R, NB * W))


def make_in_maps(inp, filt, bias):
    wconst, bconst = make_consts(filt, bias)
    inp_b = inp.astype(NP_BF16)
    return [
        {"staged_in": _stage_input(inp_b[n]),
         "wconst": wconst,
         "bconst": bconst}
        for n in range(N_CORES)
    ]


def unstage_output(staged: np.ndarray) -> np.ndarray:
    """[112, 73*1022] bf16 -> [8, 1022, 1022] f32 (m=co*14+dx, col b*1022+y
    -> out[co, 14b+dx, y])."""
    s = staged.reshape(COUT, D, NB, WOUT).transpose(0, 2, 1, 3)
    return s.reshape(COUT, HOUT, WOUT).astype(np.float32)


def kernel(inp: np.ndarray, filt: np.ndarray, bias: np.ndarray) -> np.ndarray:
    inp = np.asarray(inp, np.float32)
    filt = np.asarray(filt, np.float32)
    bias = np.asarray(bias, np.float32)
    nc = _get_nc()
    in_maps = make_in_maps(inp, filt, bias)
    res = run_bass_kernel_spmd(nc, in_maps, list(range(N_CORES)))
    return np.stack([unstage_output(res.results[c]["staged_out"])
                     for c in range(N_CORES)], axis=0)


# revision 9
# speedup vs baseline: 4.6945x; 1.0664x over previous
"""Trainium2 Bass kernel: Conv2d [8,8,1024,1024] x [8,8,3,3] (+bias), with
the reference's roll-by-1 on H, VALID padding -> [8,8,1022,1022].

Strategy: data-parallel over the batch dim (1 image per NeuronCore, 8 cores).
The kernel is HBM-bandwidth bound; all layout work is pushed to the host so
the device sees only large dense DMAs:

  - Everything on HBM is bf16 (tolerance is 2e-2; bf16 end-to-end is ~4e-3).
  - The host pre-stages the input into the exact SBUF tile layout
    `staged_in[128, 73*1024]`: partition p = c*16+q holds row 14b+q-1 (the
    roll is folded in) of channel c for block b at columns [1024b, 1024b+1024).
    Blocks are fetched 8-9 at a time -> one dense 16-18KB descriptor per
    partition per dma_start (vs 2-4KB strided descriptors straight from NCHW,
    which run at ~half rate and pay a per-dma_start completion stall).
  - Per block the conv is 3 accumulating matmuls (one per W-tap j) with the
    banded weight lhsT[(c,q),(co,dx)] = filt[co,c,q-dx,j]; K=128 = 16 rows x
    8 cin, M=112 = 14 output rows x 8 cout, N = the W dim in chunks 512+510.
  - PSUM is evicted (+bias, cast to bf16) alternately by VectorE and ScalarE
    into ot[112, g*1022]; one dense dma_start per group writes
    staged_out[112, 73*1022].  The host reassembles [8,1022,1022] f32.
"""

import os
import sys

for _p in ("/opt/trn_rl_repo",):
    if _p not in sys.path and os.path.isdir(_p):
        sys.path.insert(0, _p)

import ml_dtypes
import numpy as np

import concourse.bacc as bacc
import concourse.bass as bass
import concourse.mybir as mybir
from concourse.bass_utils import run_bass_kernel_spmd
from concourse.tile import TileContext

F32 = mybir.dt.float32
BF16 = mybir.dt.bfloat16
NP_BF16 = ml_dtypes.bfloat16

N_CORES = 8
CIN = 8
COUT = 8
KH = 3
KW = 3
H = 1024
W = 1024
HOUT = H - 2
WOUT = W - 2
D = 14                # output rows per block
R = D + 2             # input rows per block
NB = HOUT // D        # 73 blocks, exact
M = COUT * D          # 112
CHUNKS = [(0, 512), (512, 510)]
# staircase: small first groups so the PE starts ~4us in instead of waiting
# for a full 2MB prefetch; small last group to shorten the output drain.
_SIZES = [2, 3, 8, 8, 8, 8, 8, 8, 8, 8, 4]
assert sum(_SIZES) == NB
GROUPS = []
_b = 0
for _g in _SIZES:
    GROUPS.append((_b, _g))
    _b += _g
GMAX = max(_SIZES)


def build_nc(in_bufs: int = 3, out_bufs: int = 3, psum_bufs: int = 8):
    nc = bacc.Bacc("TRN2", target_bir_lowering=False, debug=False,
                   num_devices=N_CORES)
    in_d = nc.dram_tensor("staged_in", [128, NB * W], BF16,
                          kind="ExternalInput")
    w_d = nc.dram_tensor("wconst", [128, KW * M], BF16, kind="ExternalInput")
    b_d = nc.dram_tensor("bconst", [M, 1], F32, kind="ExternalInput")
    out_d = nc.dram_tensor("staged_out", [M, NB * WOUT], BF16,
                           kind="ExternalOutput")

    with TileContext(nc) as tc:
        with (
            tc.tile_pool(name="win", bufs=1) as wpool,
            tc.tile_pool(name="inp", bufs=in_bufs) as ipool,
            tc.tile_pool(name="outp", bufs=out_bufs) as opool,
            tc.tile_pool(name="ps", bufs=psum_bufs, space="PSUM") as ppool,
        ):
            wt = wpool.tile([128, KW * M], BF16, tag="wt")
            bt = wpool.tile([M, 1], F32, tag="bt")
            nc.sync.dma_start(out=wt[:], in_=w_d[:])
            nc.sync.dma_start(out=bt[:], in_=b_d[:])

            ev = 0
            for (b0, g) in GROUPS:
                t = ipool.tile([128, GMAX * W], BF16, tag="t")
                nc.sync.dma_start(
                    out=t[0:128, 0:g * W],
                    in_=in_d[:, b0 * W:(b0 + g) * W])
                ot = opool.tile([M, GMAX * WOUT], BF16, tag="ot")
                for i in range(g):
                    ps0 = ppool.tile([M, 512], F32, tag="ps")
                    ps1 = ppool.tile([M, 512], F32, tag="ps")
                    pss = [ps0, ps1]
                    for j in range(KW):  # j outer: lhsT shared by both chunks
                        for ci, (c0, n) in enumerate(CHUNKS):
                            nc.tensor.matmul(
                                pss[ci][0:M, 0:n],
                                lhsT=wt[:, j * M:(j + 1) * M],
                                rhs=t[0:128, i * W + c0 + j:
                                      i * W + c0 + j + n],
                                start=(j == 0),
                                stop=(j == KW - 1),
                            )
                    for ci, (c0, n) in enumerate(CHUNKS):
                        dst = ot[0:M, i * WOUT + c0:i * WOUT + c0 + n]
                        if ev % 2 == 0:
                            nc.vector.tensor_scalar_add(dst, pss[ci][0:M, 0:n],
                                                        bt[:])
                        else:
                            nc.scalar.add(dst, pss[ci][0:M, 0:n], bt[:])
                        ev += 1
                nc.scalar.dma_start(
                    out=out_d[:, b0 * WOUT:(b0 + g) * WOUT],
                    in_=ot[0:M, 0:g * WOUT])

    nc.compile()
    return nc


def make_consts(filt: np.ndarray, bias: np.ndarray):
    wconst = np.zeros((128, KW * M), np.float32)
    for j in range(KW):
        for q in range(R):
            for dx in range(D):
                i = q - dx
                if 0 <= i < KH:
                    for c in range(CIN):
                        wconst[c * R + q,
                               j * M + np.arange(COUT) * D + dx] = \
                            filt[:, c, i, j]
    bconst = np.repeat(bias, D).astype(np.float32)[:, None]
    return wconst.astype(NP_BF16), bconst


_CACHE = {}


def _get_nc():
    if "nc" not in _CACHE:
        _CACHE["nc"] = build_nc()
    return _CACHE["nc"]


def _stage_input(core_bf16: np.ndarray) -> np.ndarray:
    """[8,1024,1024] bf16 -> staged [128, 73*1024]: partition c*16+q, block b
    holds rolled row 14b+q = orig row (14b+q-1) mod 1024."""
    dev = np.concatenate([core_bf16[:, -1:, :], core_bf16], axis=1)  # 1025 rows
    s = np.lib.stride_tricks.as_strided(
        dev,
        shape=(CIN, R, NB, W),
        strides=(dev.strides[0], dev.strides[1], D * dev.strides[1],
                 dev.strides[2]))
    return np.ascontiguousarray(s.reshape(CIN * R, NB * W))


def make_in_maps(inp, filt, bias):
    wconst, bconst = make_consts(filt, bias)
    inp_b = inp.astype(NP_BF16)
    return [
        {"staged_in": _stage_input(inp_b[n]),
         "wconst": wconst,
         "bconst": bconst}
        for n in range(N_CORES)
    ]


def unstage_output(staged: np.ndarray) -> np.ndarray:
    """[112, 73*1022] bf16 -> [8, 1022, 1022] f32 (m=co*14+dx, col b*1022+y
    -> out[co, 14b+dx, y])."""
    s = staged.reshape(COUT, D, NB, WOUT).transpose(0, 2, 1, 3)
    return s.reshape(COUT, HOUT, WOUT).astype(np.float32)


def kernel(inp: np.ndarray, filt: np.ndarray, bias: np.ndarray) -> np.ndarray:
    inp = np.asarray(inp, np.float32)
    filt = np.asarray(filt, np.float32)
    bias = np.asarray(bias, np.float32)
    nc = _get_nc()
    in_maps = make_in_maps(inp, filt, bias)
    res = run_bass_kernel_spmd(nc, in_maps, list(range(N_CORES)))
    return np.stack([unstage_output(res.results[c]["staged_out"])
                     for c in range(N_CORES)], axis=0)
